# revision 10
# baseline (speedup 1.0000x reference)
"""Trainium2 Bass kernel for nn_Cooord_Attn (B=2,C=64,H=W=64, dual NxN attention).

Sharding: 8 cores = 2 batches x 4 query-row-quarters.

The wall clock for this problem is dominated by host->device input shipping
over the axon tunnel (~27 MB/s), not device compute (<1 ms), so the kernel is
organized to minimize shipped bytes:
  - x/guide ship as fp16 22-row slabs (16 canonical + halo rows for the 3x3
    convs), coord channels folded in; ~192 KB per slab per core.
  - The channel-attention gate (a 128-scalar sigmoid of per-channel means)
    is computed on host and shipped as 128 floats instead of shipping the
    full image twice per core (saves ~8 MB).
  - All big weights ship as ONE fp16 blob sharded 1/8 per core and
    reassembled on device with an 8-core AllGather (saves ~4 MB).
  - Conv-tail row masks ship as [1, n] rows and are partition-broadcast on
    device; output returns as fp16.

Each core:
  - computes gated coord-conv features for its 20-row query slab,
  - computes K/V/GK for its 16 canonical rows, AllGathers them within its
    4-core batch group to get the full 4096-key set,
  - runs both attentions (x and guide; both use x's values) for its 1280
    queries with softmax computed as exp(S - b*)/rowsum where b* is a
    per-attention upper bound on S (0.5*(max||q||^2 + max||k||^2)), which keys
    the whole softmax off key-major S^T tiles and avoids any transpose,
  - rowsum rides the AV matmul as a ones-column of V^T,
  - finishes the conv tail (c1/c2/sc) on its 16 output rows.
Host assembles the 8 [64,16,64] slices into (2,64,64,64).
"""
import sys
import numpy as np

sys.path.insert(0, "/opt/trn_rl_repo")

import concourse.bass as bass  # noqa: E402
import concourse.tile as tile  # noqa: E402
from concourse import bacc, mybir  # noqa: E402
from concourse.bass_utils import run_bass_kernel_spmd  # noqa: E402

F32 = mybir.dt.float32
F16 = mybir.dt.float16
AF = mybir.ActivationFunctionType
ALU = mybir.AluOpType
AX = mybir.AxisListType

B, C, H, W = 2, 64, 64, 64
N = H * W            # 4096 pixels per image
QROWS = 20           # 16 canonical + 2 halo rows each side
QN = QROWS * W       # 1280 local queries
KROWS = 16
KN = KROWS * W       # 1024 local keys
SLABR = QROWS + 2    # conv input rows = 22
PW = W + 2           # padded width 66
NT = N // 128        # 32 key tiles
HALF = QN // 2       # 640, query half per psum pass

# fp16 weight blob layout: name -> (element offset, rows, cols)
_WSIZES = [
    ("cw", 66, 9 * C),
    ("c1w", C, 9 * C),
    ("c2w", C, 9 * C),
    ("wq", C, C),
    ("wk", C, C),
    ("wgq", C, C),
    ("wgk", C, C),
    ("vtwb", 65, C),
    ("scw", C, C),
]
WOFF = {}
_off = 0
for _n, _r, _c in _WSIZES:
    WOFF[_n] = (_off, _r, _c)
    _off += _r * _c
WBLOB = _off             # 136384 elements
WSH = WBLOB // 8         # 17048 elements per core shard

# K/V AllGather buffer layout (fp32 words per rank)
OFF_K = 0
OFF_GK = 65536
OFF_VT = 131072
OFF_ST = OFF_VT + 65536          # 196608, 2 stats words
AGW = 196624                     # padded per-rank words

# packed per-core input layout (fp16 elements): everything ships as ONE
# tensor per core — per-tensor PJRT transfer overhead dominates once byte
# counts are small, so fewer tensors beats fewer bytes.
OFF_SLABX = 0
OFF_SLABG = OFF_SLABX + 66 * SLABR * PW      # 95832
OFF_W = OFF_SLABG + 66 * SLABR * PW          # 191664
OFF_BVEC = OFF_W + WSH                       # 208712
OFF_MQ = OFF_BVEC + C * 12                   # 209480
OFF_MC1 = OFF_MQ + QN                        # 210760
PKTOT = OFF_MC1 + 18 * W                     # 211912

_CACHE = {}


def _build_program():
    nc = bacc.Bacc(None, target_bir_lowering=False, debug=False, num_devices=8)

    # single packed per-core input tensor
    pk = nc.dram_tensor("pk", [PKTOT], F16, kind="ExternalInput")

    out_d = nc.dram_tensor("out", [C, KN], F16, kind="ExternalOutput")

    rg = [[0, 1, 2, 3], [4, 5, 6, 7]]
    rg8 = [[0, 1, 2, 3, 4, 5, 6, 7]]

    with tile.TileContext(nc) as tc:
        with (
            tc.tile_pool(name="const", bufs=1) as cp,
            tc.tile_pool(name="big", bufs=1) as bp,
            tc.tile_pool(name="small", bufs=2) as sp,
            tc.tile_pool(name="dram", bufs=1, space="DRAM") as dp,
        ):
            # ---- weight blob AllGather: 1/8 fp16 shard per core -> full ----
            # (collectives cannot read IO tensors; bounce through a DRAM tile)
            wag_in = dp.tile([WSH], F16)
            nc.sync.dma_start(wag_in[:], pk[OFF_W:OFF_W + WSH])
            wag = dp.tile([WBLOB], F16)
            nc.gpsimd.collective_compute(
                "AllGather", ALU.bypass, ins=[wag_in.opt()], outs=[wag.opt()],
                replica_groups=rg8)

            def wtile(name, f32=True):
                off, r, c2 = WOFF[name]
                t16 = cp.tile([r, c2], F16, tag="w16_" + name)
                nc.sync.dma_start(
                    t16[:], wag[off:off + r * c2].rearrange("(r c) -> r c", c=c2))
                if not f32:
                    return t16
                t = cp.tile([r, c2], F32, tag="w32_" + name)
                nc.vector.tensor_copy(t[:], t16[:])
                return t

            cw_s = wtile("cw", f32=False)   # fp16: multiplies the fp16 slabs
            c1w_s = wtile("c1w")
            c2w_s = wtile("c2w")
            wq_s = wtile("wq")
            wk_s = wtile("wk")
            wgq_s = wtile("wgq")
            wgk_s = wtile("wgk")
            vtwb_s = wtile("vtwb")
            scw_s = wtile("scw")

            # ---- small per-core constants (fp16 packed -> f32 tiles) ----
            bv16_s = cp.tile([C, 12], F16, tag="bv16")
            nc.sync.dma_start(
                bv16_s[:], pk[OFF_BVEC:OFF_BVEC + C * 12].rearrange("(r c) -> r c", c=12))
            bv_s = cp.tile([C, 12], F32)
            nc.vector.tensor_copy(bv_s[:], bv16_s[:])

            def bcol(idx, tag):
                t = cp.tile([C, 1], F32, tag="bc_" + tag)
                nc.vector.tensor_copy(t[:], bv_s[:, idx:idx + 1])
                return t

            bq_s = bcol(0, "bq"); bk_s = bcol(1, "bk")
            bgq_s = bcol(2, "bgq"); bgk_s = bcol(3, "bgk")
            c1b_s = bcol(4, "c1b"); c2b_s = bcol(5, "c2b"); scb_s = bcol(6, "scb")
            awx = bcol(7, "awx"); awg = bcol(8, "awg")
            al64_s = bcol(9, "al64")
            gam1_s = cp.tile([1, 1], F32, tag="gam1")
            nc.vector.tensor_copy(gam1_s[:], bv_s[0:1, 10:11])

            mq16_s = cp.tile([1, QN], F16, tag="mq16")
            nc.sync.dma_start(
                mq16_s[:], pk[OFF_MQ:OFF_MQ + QN].rearrange("(o c) -> o c", o=1))
            maskq_s = cp.tile([1, QN], F32)
            nc.vector.tensor_copy(maskq_s[:], mq16_s[:])
            mrow_s = cp.tile([1, 18 * W], F16, tag="mrow16")
            nc.sync.dma_start(
                mrow_s[:], pk[OFF_MC1:OFF_MC1 + 18 * W].rearrange("(o c) -> o c", o=1))
            mrow32_s = cp.tile([1, 18 * W], F32, tag="mrow32")
            nc.vector.tensor_copy(mrow32_s[:], mrow_s[:])
            mc1_s = cp.tile([C, 18 * W], F32)
            nc.gpsimd.partition_broadcast(mc1_s[:], mrow32_s[0:1, :])
            ones64 = cp.tile([C, 1], F32); nc.vector.memset(ones64[:], 1.0)

            cs_s = bp.tile([66, SLABR * PW], F16)
            nc.sync.dma_start(
                cs_s[:],
                pk[OFF_SLABX:OFF_SLABX + 66 * SLABR * PW].rearrange(
                    "(r c) -> r c", c=SLABR * PW))
            gs_s = bp.tile([66, SLABR * PW], F16)
            nc.sync.dma_start(
                gs_s[:],
                pk[OFF_SLABG:OFF_SLABG + 66 * SLABR * PW].rearrange(
                    "(r c) -> r c", c=SLABR * PW))

            # ---- coord conv -> gated features xgt/ggt [65, QN] (row 64 = ones) ----
            xgt = bp.tile([65, QN], F32)
            ggt = bp.tile([65, QN], F32)
            nc.vector.memset(xgt[64:65, :], 1.0)
            nc.vector.memset(ggt[64:65, :], 1.0)

            with tc.tile_pool(name="feps", bufs=3, space="PSUM") as fp:
                def coord_conv(slab_s, aw, dst):
                    for r0, nr in ((0, 8), (8, 8), (16, 4)):
                        ps = fp.tile([C, 512], F32, tag="fe_ps")
                        slab3 = slab_s[:].rearrange("c (r w) -> c r w", w=PW)
                        for dy in range(3):
                            for dx in range(3):
                                nc.tensor.matmul(
                                    ps[:, 0:nr * W],
                                    cw_s[:, (dy * 3 + dx) * C:(dy * 3 + dx + 1) * C],
                                    slab3[:, r0 + dy:r0 + dy + nr, dx:dx + W],
                                    start=(dy == 0 and dx == 0),
                                    stop=(dy == 2 and dx == 2),
                                )
                        nc.vector.tensor_scalar_mul(
                            dst[0:C, r0 * W:(r0 + nr) * W], ps[:, 0:nr * W], aw[:, 0:1])

                coord_conv(cs_s, awx, xgt)
                coord_conv(gs_s, awg, ggt)

                # ---- 1x1 projections ----
                qx = bp.tile([C, QN], F32)
                gqx = bp.tile([C, QN], F32)
                kx = bp.tile([C, KN], F32)
                gkx = bp.tile([C, KN], F32)

                def lin(src, w_s, b_s, dst, off, n):
                    c0 = 0
                    while c0 < n:
                        cw_ = min(512, n - c0)
                        ps = fp.tile([C, 512], F32, tag="fe_ps")
                        nc.tensor.matmul(ps[:, 0:cw_], w_s[:], src[0:C, off + c0:off + c0 + cw_],
                                         start=True, stop=True)
                        nc.vector.tensor_scalar_add(dst[:, c0:c0 + cw_], ps[:, 0:cw_], b_s[:, 0:1])
                        c0 += cw_

                lin(xgt, wq_s, bq_s, qx, 0, QN)
                lin(ggt, wgq_s, bgq_s, gqx, 0, QN)
                lin(xgt, wk_s, bk_s, kx, 2 * W, KN)
                lin(ggt, wgk_s, bgk_s, gkx, 2 * W, KN)

                # V^T local [128, 8*64]
                vt_loc = bp.tile([128, 8 * C], F32)
                for t in range(8):
                    ps = fp.tile([128, C], F32, tag="fe_ps")
                    nc.tensor.matmul(ps[:], xgt[:, 2 * W + 128 * t:2 * W + 128 * (t + 1)],
                                     vtwb_s[:], start=True, stop=True)
                    nc.vector.tensor_copy(vt_loc[:, C * t:C * (t + 1)], ps[:])

                # ---- local k^2 stats (max over keys of sum_c k^2) ----
                def sq_colmax(src, n, tagp):
                    sq = bp.tile([C, QN], F32, tag="sq_tmp")
                    nc.vector.tensor_mul(sq[:, 0:n], src[:, 0:n], src[:, 0:n])
                    parts = sp.tile([1, 4], F32, tag=tagp + "_p")
                    c0, idx = 0, 0
                    while c0 < n:
                        cw_ = min(512, n - c0)
                        ps = fp.tile([1, 512], F32, tag="fe_ps")
                        nc.tensor.matmul(ps[:, 0:cw_], ones64[:], sq[0:C, c0:c0 + cw_],
                                         start=True, stop=True)
                        nc.vector.reduce_max(parts[:, idx:idx + 1], ps[0:1, 0:cw_], axis=AX.X)
                        c0 += cw_
                        idx += 1
                    while idx < 4:
                        nc.vector.tensor_copy(parts[:, idx:idx + 1], parts[:, 0:1])
                        idx += 1
                    mx = sp.tile([1, 1], F32, tag=tagp)
                    nc.vector.reduce_max(mx[:], parts[0:1, :], axis=AX.X)
                    return mx

                k2x = sq_colmax(kx, KN, "k2x")
                k2g = sq_colmax(gkx, KN, "k2g")
                q2x = sq_colmax(qx, QN, "q2x")
                q2g = sq_colmax(gqx, QN, "q2g")

            # ---- AllGather K/GK/V^T/stats within batch group ----
            stats = sp.tile([1, 2], F32, tag="stats")
            nc.vector.tensor_copy(stats[:, 0:1], k2x[:])
            nc.vector.tensor_copy(stats[:, 1:2], k2g[:])

            agin = dp.tile([AGW], F32)
            agout = dp.tile([4, AGW], F32)
            nc.sync.dma_start(agin[OFF_K:OFF_K + C * KN].rearrange("(c m) -> c m", m=KN), kx[:])
            nc.sync.dma_start(agin[OFF_GK:OFF_GK + C * KN].rearrange("(c m) -> c m", m=KN), gkx[:])
            nc.sync.dma_start(
                agin[OFF_VT:OFF_VT + 65536].rearrange("(t p c) -> p t c", p=128, c=C),
                vt_loc[:].rearrange("p (t c) -> p t c", c=C))
            nc.sync.dma_start(agin[OFF_ST:OFF_ST + 2].rearrange("(o s) -> o s", o=1), stats[:])
            nc.gpsimd.collective_compute(
                "AllGather", ALU.bypass, ins=[agin.opt()], outs=[agout.opt()],
                replica_groups=rg)

            kf = bp.tile([C, N], F32, tag="bigA")
            gkf = bp.tile([C, N], F32, tag="bigB")
            vtf = bp.tile([128, NT * 65], F32)
            statsf = sp.tile([1, 8], F32, tag="statsf")
            nc.sync.dma_start(
                kf[:].rearrange("c (r m) -> c r m", m=KN),
                agout[:, OFF_K:OFF_K + C * KN].rearrange("r (c m) -> c r m", m=KN))
            nc.sync.dma_start(
                gkf[:].rearrange("c (r m) -> c r m", m=KN),
                agout[:, OFF_GK:OFF_GK + C * KN].rearrange("r (c m) -> c r m", m=KN))
            vtf4 = vtf[:].rearrange("p (u e) -> p u e", e=65)
            for r in range(4):
                nc.sync.dma_start(
                    vtf4[:, 8 * r:8 * r + 8, 0:C],
                    agout[r, OFF_VT:OFF_VT + 65536].rearrange("(t p c) -> p t c", p=128, c=C))
            nc.vector.memset(vtf4[:, :, C:65], 1.0)
            nc.sync.dma_start(
                statsf[:].rearrange("o (r s) -> o r s", s=2),
                agout[None, :, OFF_ST:OFF_ST + 2])

            # global key maxes and exp biases
            kmax = sp.tile([1, 2], F32, tag="kmax")
            nc.vector.reduce_max(kmax[:], statsf[0:1, :].rearrange("o (r s) -> o s r", s=2),
                                 axis=AX.X)

            def mk_bias(q2, koff):
                t = sp.tile([1, 1], F32, tag="bias_t" + str(koff))
                nc.vector.tensor_add(t[:], q2[:], kmax[:, koff:koff + 1])
                nc.vector.tensor_scalar_mul(t[:], t[:], -0.5)
                col = cp.tile([128, 1], F32, tag="bias_col" + str(koff))
                nc.gpsimd.partition_broadcast(col[:], t[0:1, :])
                return col

            bias_x = mk_bias(q2x, 0)
            bias_g = mk_bias(q2g, 1)

            # ---- attention (guide first, then x) ----
            ong = bp.tile([C, QN], F32)    # raw guide_out (masked, unscaled)
            ocx = bp.tile([C, QN], F32)    # gamma * x_out (masked)

            with (
                tc.tile_pool(name="aps_s", bufs=2, space="PSUM") as pss,
                tc.tile_pool(name="aps_o", bufs=2, space="PSUM") as pso,
                tc.tile_pool(name="atp", bufs=3) as atp,
            ):
                for (q_t, kf_t, bias_c, dst, gscale) in (
                    (gqx, gkf, bias_g, ong, None),
                    (qx, kf, bias_x, ocx, gam1_s),
                ):
                    for h in range(2):
                        o = pso.tile([65, HALF], F32, tag="o_ps")
                        for t in range(NT):
                            s = pss.tile([128, HALF], F32, tag="s_ps")
                            nc.tensor.matmul(s[:, 0:512], kf_t[:, 128 * t:128 * (t + 1)],
                                             q_t[:, HALF * h:HALF * h + 512],
                                             start=True, stop=True)
                            nc.tensor.matmul(s[:, 512:HALF], kf_t[:, 128 * t:128 * (t + 1)],
                                             q_t[:, HALF * h + 512:HALF * (h + 1)],
                                             start=True, stop=True)
                            at = atp.tile([128, HALF], F32, tag="at")
                            nc.scalar.activation(at[:], s[:], AF.Exp, bias=bias_c[:, 0:1],
                                                 scale=1.0)
                            nc.tensor.matmul(o[:, 0:512], vtf4[:, t, :], at[:, 0:512],
                                             start=(t == 0), stop=(t == NT - 1))
                            nc.tensor.matmul(o[:, 512:HALF], vtf4[:, t, :], at[:, 512:HALF],
                                             start=(t == 0), stop=(t == NT - 1))
                        rs = sp.tile([1, HALF], F32, tag="rs")
                        nc.vector.tensor_copy(rs[:], o[C:65, :])
                        rc = sp.tile([1, HALF], F32, tag="rc")
                        nc.vector.reciprocal(rc[:], rs[:])
                        nc.vector.tensor_mul(rc[:], rc[:], maskq_s[0:1, HALF * h:HALF * (h + 1)])
                        if gscale is not None:
                            nc.vector.tensor_scalar_mul(rc[:], rc[:], gscale[0:1, 0:1])
                        rb = sp.tile([C, HALF], F32, tag="rb")
                        nc.gpsimd.partition_broadcast(rb[:], rc[0:1, :])
                        nc.vector.tensor_mul(dst[:, HALF * h:HALF * (h + 1)], o[0:C, :], rb[:])

            # ---- combine + conv tail ----
            oc = bp.tile([C, QN], F32)
            talbum = bp.tile([C, QN], F32)
            nc.vector.tensor_scalar_mul(talbum[:], ong[:], al64_s[:, 0:1])
            nc.vector.tensor_add(oc[:], ocx[:], talbum[:])

            lks = bp.tile([C, QROWS * PW], F32)
            nc.vector.memset(lks[:], 0.0)
            lks3 = lks[:].rearrange("c (r w) -> c r w", w=PW)
            oc3 = oc[:].rearrange("c (r w) -> c r w", w=W)
            nc.vector.tensor_scalar_mul(talbum[:], oc[:], 0.1)
            nc.vector.tensor_max(lks3[:, :, 1:1 + W], oc3[:],
                                 talbum[:].rearrange("c (r w) -> c r w", w=W))

            c1s = bp.tile([C, QROWS * PW], F32)
            nc.vector.memset(c1s[:], 0.0)
            c1s3 = c1s[:].rearrange("c (r w) -> c r w", w=PW)
            mc13 = mc1_s[:].rearrange("c (r w) -> c r w", w=W)

            with tc.tile_pool(name="beps", bufs=3, space="PSUM") as bps:
                # c1 on slab rows [1,19)
                for r0, nr in ((1, 8), (9, 8), (17, 2)):
                    ps = bps.tile([C, 512], F32, tag="be_ps")
                    for dy in range(3):
                        for dx in range(3):
                            nc.tensor.matmul(
                                ps[:, 0:nr * W],
                                c1w_s[:, (dy * 3 + dx) * C:(dy * 3 + dx + 1) * C],
                                lks3[:, r0 + dy - 1:r0 + dy - 1 + nr, dx:dx + W],
                                start=(dy == 0 and dx == 0), stop=(dy == 2 and dx == 2))
                    tmp = sp.tile([C, 512], F32, tag="c1_tmp")
                    nc.vector.tensor_scalar_add(tmp[:, 0:nr * W], ps[:, 0:nr * W], c1b_s[:, 0:1])
                    tmp2 = sp.tile([C, 512], F32, tag="c1_tmp2")
                    nc.vector.tensor_scalar_mul(tmp2[:, 0:nr * W], tmp[:, 0:nr * W], 0.1)
                    nc.vector.tensor_max(tmp[:, 0:nr * W], tmp[:, 0:nr * W], tmp2[:, 0:nr * W])
                    nc.vector.tensor_mul(
                        c1s3[:, r0:r0 + nr, 1:1 + W],
                        tmp[:, 0:nr * W].rearrange("c (r w) -> c r w", w=W),
                        mc13[:, r0 - 1:r0 - 1 + nr, :])

                # c2 on slab rows [2,18) -> branch [C, KN]
                branch = bp.tile([C, KN], F32)
                for r0, nr in ((2, 8), (10, 8)):
                    ps = bps.tile([C, 512], F32, tag="be_ps")
                    for dy in range(3):
                        for dx in range(3):
                            nc.tensor.matmul(
                                ps[:, 0:nr * W],
                                c2w_s[:, (dy * 3 + dx) * C:(dy * 3 + dx + 1) * C],
                                c1s3[:, r0 + dy - 1:r0 + dy - 1 + nr, dx:dx + W],
                                start=(dy == 0 and dx == 0), stop=(dy == 2 and dx == 2))
                    nc.vector.tensor_scalar_add(branch[:, (r0 - 2) * W:(r0 - 2 + nr) * W],
                                                ps[:, 0:nr * W], c2b_s[:, 0:1])

                # sc 1x1 on oc rows [2,18), final = branch + sc * guide_out (fp16 out)
                finalv = bp.tile([C, KN], F16)
                for c0 in (0, 512):
                    ps = bps.tile([C, 512], F32, tag="be_ps")
                    nc.tensor.matmul(ps[:], scw_s[:], oc[:, 2 * W + c0:2 * W + c0 + 512],
                                     start=True, stop=True)
                    tmp = sp.tile([C, 512], F32, tag="sc_tmp")
                    nc.vector.tensor_scalar_add(tmp[:], ps[:], scb_s[:, 0:1])
                    nc.vector.tensor_mul(tmp[:], tmp[:], ong[:, 2 * W + c0:2 * W + c0 + 512])
                    nc.vector.tensor_add(finalv[:, c0:c0 + 512], branch[:, c0:c0 + 512], tmp[:])

                nc.sync.dma_start(out_d[:], finalv[:])

    nc.compile()
    return nc


def _host_inputs(inputs):
    """Build the 8 per-core input maps from the full problem inputs."""
    x = np.asarray(inputs["x"], np.float32)
    guide = np.asarray(inputs["guide"], np.float32)
    lin_w = float(np.asarray(inputs["lin_w"]))
    lin_b = float(np.asarray(inputs["lin_b"]))
    coord_w = np.asarray(inputs["coord_w"], np.float32)   # (64, 66, 3, 3)
    gamma = float(np.asarray(inputs["gamma"]).reshape(-1)[0])
    alpha = float(np.asarray(inputs["alpha"]).reshape(-1)[0])

    # channel attention: sigmoid(lw*leaky(lw*mean + lb) + lb), 64 scalars/batch
    def aw_host(t):
        p = t.mean(axis=(2, 3), dtype=np.float64) * lin_w + lin_b
        h = np.where(p > 0, p, 0.1 * p) * lin_w + lin_b
        return (1.0 / (1.0 + np.exp(-h))).astype(np.float32)   # (B, C)

    awx = aw_host(x)
    awg = aw_host(guide)

    # coordinate channels
    xx = (np.arange(W, dtype=np.float32) / (W - 1)) * 2 - 1
    yy = (np.arange(H, dtype=np.float32) / (H - 1)) * 2 - 1

    def taps(w):  # (O, I, 3, 3) -> [I, 9*O], tap-major
        o, i = w.shape[0], w.shape[1]
        out = np.zeros((i, 9 * o), np.float32)
        for dy in range(3):
            for dx in range(3):
                out[:, (dy * 3 + dx) * o:(dy * 3 + dx + 1) * o] = w[:, :, dy, dx].T
        return out

    wT = lambda k: np.asarray(inputs[k], np.float32).T
    bc = lambda k: np.asarray(inputs[k], np.float32).reshape(C)

    vtwb = np.concatenate(
        [wT("xv_w"), np.asarray(inputs["xv_b"], np.float32).reshape(1, C)], axis=0)

    blob = np.concatenate([
        taps(coord_w).ravel(),
        taps(np.asarray(inputs["c1_w"], np.float32)).ravel(),
        taps(np.asarray(inputs["c2_w"], np.float32)).ravel(),
        wT("xq_w").ravel(), wT("xk_w").ravel(),
        wT("gq_w").ravel(), wT("gk_w").ravel(),
        vtwb.ravel(), wT("sc_w").ravel(),
    ]).astype(np.float16).reshape(8, WSH)

    bvec_b = []
    for b in range(B):
        v = np.zeros((C, 12), np.float16)
        v[:, 0] = bc("xq_b"); v[:, 1] = bc("xk_b")
        v[:, 2] = bc("gq_b"); v[:, 3] = bc("gk_b")
        v[:, 4] = bc("c1_b"); v[:, 5] = bc("c2_b"); v[:, 6] = bc("sc_b")
        v[:, 7] = awx[b]; v[:, 8] = awg[b]
        v[:, 9] = alpha; v[:, 10] = gamma
        bvec_b.append(v)

    in_maps = []
    for i in range(8):
        b, j = divmod(i, 4)
        pkv = np.zeros(PKTOT, np.float16)
        cs = pkv[OFF_SLABX:OFF_SLABG].reshape(66, SLABR, PW)
        gs = pkv[OFF_SLABG:OFF_W].reshape(66, SLABR, PW)

        r_lo = 16 * j - 3                      # slab image rows [r_lo, r_lo+22)
        lo, hi = max(0, r_lo), min(H, r_lo + SLABR)
        s0, s1 = lo - r_lo, hi - r_lo
        cs[0:C, s0:s1, 1:1 + W] = x[b, :, lo:hi, :]
        gs[0:C, s0:s1, 1:1 + W] = guide[b, :, lo:hi, :]
        cs[C, s0:s1, 1:1 + W] = xx
        cs[C + 1, s0:s1, 1:1 + W] = yy[lo:hi, None]
        gs[C, s0:s1, 1:1 + W] = xx
        gs[C + 1, s0:s1, 1:1 + W] = yy[lo:hi, None]

        pkv[OFF_W:OFF_BVEC] = blob[i]
        pkv[OFF_BVEC:OFF_MQ] = bvec_b[b].ravel()

        q_lo = 16 * j - 2
        mq = pkv[OFF_MQ:OFF_MC1].reshape(QROWS, W)
        mq[max(0, -q_lo):QROWS - max(0, q_lo + QROWS - H)] = 1.0
        c_lo = 16 * j - 1
        mc = pkv[OFF_MC1:PKTOT].reshape(18, W)
        mc[max(0, -c_lo):18 - max(0, c_lo + 18 - H)] = 1.0

        in_maps.append(dict(pk=pkv))
    return in_maps


def kernel(**inputs):
    if "nc" not in _CACHE:
        _CACHE["nc"] = _build_program()
    nc = _CACHE["nc"]
    in_maps = _host_inputs(inputs)
    res = run_bass_kernel_spmd(nc, in_maps, core_ids=list(range(8)))
    out = np.zeros((B, C, H, W), np.float32)
    for i in range(8):
        b, j = divmod(i, 4)
        out[b, :, 16 * j:16 * j + 16, :] = (
            res.results[i]["out"].astype(np.float32).reshape(C, KROWS, W))
    return out


# revision 19
# speedup vs baseline: 1.5019x; 1.5019x over previous
"""Trainium2 Bass kernel for nn_Cooord_Attn (B=2,C=64,H=W=64, dual NxN attention).

Sharding: 8 cores = 2 batches x 4 query-row-quarters.

The wall clock for this problem is dominated by host->device input shipping
over the axon tunnel (~27 MB/s), not device compute (<1 ms), so the kernel is
organized to minimize shipped bytes:
  - x/guide ship as fp16 22-row slabs (16 canonical + halo rows for the 3x3
    convs), coord channels folded in; ~192 KB per slab per core.
  - The channel-attention gate (a 128-scalar sigmoid of per-channel means)
    is computed on host and shipped as 128 floats instead of shipping the
    full image twice per core (saves ~8 MB).
  - All big weights ship as ONE fp16 blob sharded 1/8 per core and
    reassembled on device with an 8-core AllGather (saves ~4 MB).
  - Conv-tail row masks ship as [1, n] rows and are partition-broadcast on
    device; output returns as fp16.

Each core:
  - computes gated coord-conv features for its 20-row query slab,
  - computes K/V/GK for its 16 canonical rows, AllGathers them within its
    4-core batch group to get the full 4096-key set,
  - runs both attentions (x and guide; both use x's values) for its 1280
    queries with softmax computed as exp(S - b*)/rowsum where b* is a
    per-attention upper bound on S (0.5*(max||q||^2 + max||k||^2)), which keys
    the whole softmax off key-major S^T tiles and avoids any transpose,
  - rowsum rides the AV matmul as a ones-column of V^T,
  - finishes the conv tail (c1/c2/sc) on its 16 output rows.
Host assembles the 8 [64,16,64] slices into (2,64,64,64).
"""
import sys
import numpy as np

sys.path.insert(0, "/opt/trn_rl_repo")

import concourse.bass as bass  # noqa: E402
import concourse.tile as tile  # noqa: E402
from concourse import bacc, mybir  # noqa: E402
from concourse.bass_utils import run_bass_kernel_spmd  # noqa: E402

F32 = mybir.dt.float32
F16 = mybir.dt.float16
AF = mybir.ActivationFunctionType
ALU = mybir.AluOpType
AX = mybir.AxisListType

B, C, H, W = 2, 64, 64, 64
N = H * W            # 4096 pixels per image
QROWS = 20           # 16 canonical + 2 halo rows each side
QN = QROWS * W       # 1280 local queries
KROWS = 16
KN = KROWS * W       # 1024 local keys
SLABR = QROWS + 2    # conv input rows = 22
PW = W + 2           # padded width 66
NT = N // 128        # 32 key tiles
HALF = QN // 2       # 640, query half per psum pass

# fp16 weight blob layout: name -> (element offset, rows, cols)
_WSIZES = [
    ("cw", 66, 9 * C),
    ("c1w", C, 9 * C),
    ("c2w", C, 9 * C),
    ("wq", C, C),
    ("wk", C, C),
    ("wgq", C, C),
    ("wgk", C, C),
    ("vtwb", 65, C),
    ("scw", C, C),
]
WOFF = {}
_off = 0
for _n, _r, _c in _WSIZES:
    WOFF[_n] = (_off, _r, _c)
    _off += _r * _c
WBLOB = _off             # 136384 elements
# weights ship as contiguous 1/4-blob slices (core i carries quarter i%4) and
# are reassembled with a 4-core AllGather: 8-core collectives and DRAM->DRAM
# DMAs each cost ~50ms of fixed overhead in this runtime, 4-core ones are free.
WSH = WBLOB // 4         # 34096 elements per core shard
WSTG = (16, WBLOB // 4 // 16)   # SBUF staging shape for the shard

# K/V AllGather buffer layout (fp32 words per rank)
OFF_K = 0
OFF_GK = 65536
OFF_VT = 131072
OFF_ST = OFF_VT + 65536          # 196608, 2 stats words
AGW = 196624                     # padded per-rank words

# packed per-core input layout (fp16 elements): everything ships as ONE
# tensor per core — per-tensor PJRT transfer overhead dominates once byte
# counts are small, so fewer tensors beats fewer bytes.
OFF_SLABX = 0
OFF_SLABG = OFF_SLABX + 66 * SLABR * PW      # 95832
OFF_W = OFF_SLABG + 66 * SLABR * PW          # 191664
OFF_BVEC = OFF_W + WSH                       # 208712
OFF_MQ = OFF_BVEC + C * 12                   # 209480
OFF_MC1 = OFF_MQ + QN                        # 210760
PKTOT = OFF_MC1 + 18 * W                     # 211912

_CACHE = {}


def _build_program():
    nc = bacc.Bacc(None, target_bir_lowering=False, debug=False, num_devices=8)

    # single packed per-core input tensor
    pk = nc.dram_tensor("pk", [PKTOT], F16, kind="ExternalInput")

    out_d = nc.dram_tensor("out", [C, KN], F16, kind="ExternalOutput")

    rg = [[0, 1, 2, 3], [4, 5, 6, 7]]

    with tile.TileContext(nc) as tc:
        with (
            tc.tile_pool(name="const", bufs=1) as cp,
            tc.tile_pool(name="big", bufs=1) as bp,
            tc.tile_pool(name="small", bufs=2) as sp,
            tc.tile_pool(name="dram", bufs=1, space="DRAM") as dp,
        ):
            # ---- weight blob AllGather: 1/4 fp16 shard per core -> full ----
            # (collectives cannot read IO tensors; bounce through SBUF)
            wstg = cp.tile(list(WSTG), F16, tag="wstg")
            nc.sync.dma_start(
                wstg[:], pk[OFF_W:OFF_W + WSH].rearrange("(r c) -> r c", c=WSTG[1]))
            wag_in = dp.tile([WSH], F16)
            nc.sync.dma_start(
                wag_in[:].rearrange("(r c) -> r c", c=WSTG[1]), wstg[:])
            wag = dp.tile([WBLOB], F16)
            nc.gpsimd.collective_compute(
                "AllGather", ALU.bypass, ins=[wag_in.opt()], outs=[wag.opt()],
                replica_groups=rg)

            def wtile(name, f32=True):
                off, r, c2 = WOFF[name]
                t16 = cp.tile([r, c2], F16, tag="w16_" + name)
                nc.sync.dma_start(
                    t16[:], wag[off:off + r * c2].rearrange("(r c) -> r c", c=c2))
                if not f32:
                    return t16
                t = cp.tile([r, c2], F32, tag="w32_" + name)
                nc.vector.tensor_copy(t[:], t16[:])
                return t

            cw_s = wtile("cw", f32=False)   # fp16: multiplies the fp16 slabs
            c1w_s = wtile("c1w")
            c2w_s = wtile("c2w")
            wq_s = wtile("wq")
            wk_s = wtile("wk")
            wgq_s = wtile("wgq")
            wgk_s = wtile("wgk")
            vtwb_s = wtile("vtwb")
            scw_s = wtile("scw")

            # ---- small per-core constants (fp16 packed -> f32 tiles) ----
            bv16_s = cp.tile([C, 12], F16, tag="bv16")
            nc.sync.dma_start(
                bv16_s[:], pk[OFF_BVEC:OFF_BVEC + C * 12].rearrange("(r c) -> r c", c=12))
            bv_s = cp.tile([C, 12], F32)
            nc.vector.tensor_copy(bv_s[:], bv16_s[:])

            def bcol(idx, tag):
                t = cp.tile([C, 1], F32, tag="bc_" + tag)
                nc.vector.tensor_copy(t[:], bv_s[:, idx:idx + 1])
                return t

            bq_s = bcol(0, "bq"); bk_s = bcol(1, "bk")
            bgq_s = bcol(2, "bgq"); bgk_s = bcol(3, "bgk")
            c1b_s = bcol(4, "c1b"); c2b_s = bcol(5, "c2b"); scb_s = bcol(6, "scb")
            awx = bcol(7, "awx"); awg = bcol(8, "awg")
            al64_s = bcol(9, "al64")
            gam1_s = cp.tile([1, 1], F32, tag="gam1")
            nc.vector.tensor_copy(gam1_s[:], bv_s[0:1, 10:11])

            mq16_s = cp.tile([1, QN], F16, tag="mq16")
            nc.sync.dma_start(
                mq16_s[:], pk[OFF_MQ:OFF_MQ + QN].rearrange("(o c) -> o c", o=1))
            maskq_s = cp.tile([1, QN], F32)
            nc.vector.tensor_copy(maskq_s[:], mq16_s[:])
            mrow_s = cp.tile([1, 18 * W], F16, tag="mrow16")
            nc.sync.dma_start(
                mrow_s[:], pk[OFF_MC1:OFF_MC1 + 18 * W].rearrange("(o c) -> o c", o=1))
            mrow32_s = cp.tile([1, 18 * W], F32, tag="mrow32")
            nc.vector.tensor_copy(mrow32_s[:], mrow_s[:])
            mc1_s = cp.tile([C, 18 * W], F32)
            ones64 = cp.tile([C, 1], F32); nc.vector.memset(ones64[:], 1.0)
            # row vector of ones: stationary operand for PE-based partition
            # broadcasts (out[M,N] = ones[1,M].T @ row[1,N]) — gpsimd
            # partition_broadcast costs ~35ms of fixed ucode spin-up per call.
            ones_st = cp.tile([1, 128], F32, tag="ones_st")
            nc.vector.memset(ones_st[:], 1.0)

            cs_s = bp.tile([66, SLABR * PW], F16)
            nc.sync.dma_start(
                cs_s[:],
                pk[OFF_SLABX:OFF_SLABX + 66 * SLABR * PW].rearrange(
                    "(r c) -> r c", c=SLABR * PW))
            gs_s = bp.tile([66, SLABR * PW], F16)
            nc.sync.dma_start(
                gs_s[:],
                pk[OFF_SLABG:OFF_SLABG + 66 * SLABR * PW].rearrange(
                    "(r c) -> r c", c=SLABR * PW))

            # ---- coord conv -> gated features xgt/ggt [65, QN] (row 64 = ones) ----
            xgt = bp.tile([65, QN], F32)
            ggt = bp.tile([65, QN], F32)
            nc.vector.memset(xgt[64:65, :], 1.0)
            nc.vector.memset(ggt[64:65, :], 1.0)

            with tc.tile_pool(name="feps", bufs=3, space="PSUM") as fp:
                def coord_conv(slab_s, aw, dst):
                    for r0, nr in ((0, 8), (8, 8), (16, 4)):
                        ps = fp.tile([C, 512], F32, tag="fe_ps")
                        slab3 = slab_s[:].rearrange("c (r w) -> c r w", w=PW)
                        for dy in range(3):
                            for dx in range(3):
                                nc.tensor.matmul(
                                    ps[:, 0:nr * W],
                                    cw_s[:, (dy * 3 + dx) * C:(dy * 3 + dx + 1) * C],
                                    slab3[:, r0 + dy:r0 + dy + nr, dx:dx + W],
                                    start=(dy == 0 and dx == 0),
                                    stop=(dy == 2 and dx == 2),
                                )
                        nc.vector.tensor_scalar_mul(
                            dst[0:C, r0 * W:(r0 + nr) * W], ps[:, 0:nr * W], aw[:, 0:1])

                coord_conv(cs_s, awx, xgt)
                coord_conv(gs_s, awg, ggt)

                # ---- 1x1 projections ----
                qx = bp.tile([C, QN], F32)
                gqx = bp.tile([C, QN], F32)
                kx = bp.tile([C, KN], F32)
                gkx = bp.tile([C, KN], F32)

                def lin(src, w_s, b_s, dst, off, n):
                    c0 = 0
                    while c0 < n:
                        cw_ = min(512, n - c0)
                        ps = fp.tile([C, 512], F32, tag="fe_ps")
                        nc.tensor.matmul(ps[:, 0:cw_], w_s[:], src[0:C, off + c0:off + c0 + cw_],
                                         start=True, stop=True)
                        nc.vector.tensor_scalar_add(dst[:, c0:c0 + cw_], ps[:, 0:cw_], b_s[:, 0:1])
                        c0 += cw_

                lin(xgt, wq_s, bq_s, qx, 0, QN)
                lin(ggt, wgq_s, bgq_s, gqx, 0, QN)
                lin(xgt, wk_s, bk_s, kx, 2 * W, KN)
                lin(ggt, wgk_s, bgk_s, gkx, 2 * W, KN)

                # V^T local [128, 8*64]
                vt_loc = bp.tile([128, 8 * C], F32)
                for t in range(8):
                    ps = fp.tile([128, C], F32, tag="fe_ps")
                    nc.tensor.matmul(ps[:], xgt[:, 2 * W + 128 * t:2 * W + 128 * (t + 1)],
                                     vtwb_s[:], start=True, stop=True)
                    nc.vector.tensor_copy(vt_loc[:, C * t:C * (t + 1)], ps[:])

                # ---- local k^2 stats (max over keys of sum_c k^2) ----
                def sq_colmax(src, n, tagp):
                    sq = bp.tile([C, QN], F32, tag="sq_tmp")
                    nc.vector.tensor_mul(sq[:, 0:n], src[:, 0:n], src[:, 0:n])
                    parts = sp.tile([1, 4], F32, tag=tagp + "_p")
                    c0, idx = 0, 0
                    while c0 < n:
                        cw_ = min(512, n - c0)
                        ps = fp.tile([1, 512], F32, tag="fe_ps")
                        nc.tensor.matmul(ps[:, 0:cw_], ones64[:], sq[0:C, c0:c0 + cw_],
                                         start=True, stop=True)
                        nc.vector.reduce_max(parts[:, idx:idx + 1], ps[0:1, 0:cw_], axis=AX.X)
                        c0 += cw_
                        idx += 1
                    while idx < 4:
                        nc.vector.tensor_copy(parts[:, idx:idx + 1], parts[:, 0:1])
                        idx += 1
                    mx = sp.tile([1, 1], F32, tag=tagp)
                    nc.vector.reduce_max(mx[:], parts[0:1, :], axis=AX.X)
                    return mx

                k2x = sq_colmax(kx, KN, "k2x")
                k2g = sq_colmax(gkx, KN, "k2g")
                q2x = sq_colmax(qx, QN, "q2x")
                q2g = sq_colmax(gqx, QN, "q2g")

            # ---- AllGather K/GK/V^T/stats within batch group ----
            stats = sp.tile([1, 2], F32, tag="stats")
            nc.vector.tensor_copy(stats[:, 0:1], k2x[:])
            nc.vector.tensor_copy(stats[:, 1:2], k2g[:])

            agin = dp.tile([AGW], F32)
            agout = dp.tile([4, AGW], F32)
            nc.sync.dma_start(agin[OFF_K:OFF_K + C * KN].rearrange("(c m) -> c m", m=KN), kx[:])
            nc.sync.dma_start(agin[OFF_GK:OFF_GK + C * KN].rearrange("(c m) -> c m", m=KN), gkx[:])
            nc.sync.dma_start(
                agin[OFF_VT:OFF_VT + 65536].rearrange("(t p c) -> p t c", p=128, c=C),
                vt_loc[:].rearrange("p (t c) -> p t c", c=C))
            nc.sync.dma_start(agin[OFF_ST:OFF_ST + 2].rearrange("(o s) -> o s", o=1), stats[:])
            nc.gpsimd.collective_compute(
                "AllGather", ALU.bypass, ins=[agin.opt()], outs=[agout.opt()],
                replica_groups=rg)

            kf = bp.tile([C, N], F32, tag="bigA")
            gkf = bp.tile([C, N], F32, tag="bigB")
            vtf = bp.tile([128, NT * 65], F32)
            statsf = sp.tile([1, 8], F32, tag="statsf")
            nc.sync.dma_start(
                kf[:].rearrange("c (r m) -> c r m", m=KN),
                agout[:, OFF_K:OFF_K + C * KN].rearrange("r (c m) -> c r m", m=KN))
            nc.sync.dma_start(
                gkf[:].rearrange("c (r m) -> c r m", m=KN),
                agout[:, OFF_GK:OFF_GK + C * KN].rearrange("r (c m) -> c r m", m=KN))
            vtf4 = vtf[:].rearrange("p (u e) -> p u e", e=65)
            for r in range(4):
                nc.sync.dma_start(
                    vtf4[:, 8 * r:8 * r + 8, 0:C],
                    agout[r, OFF_VT:OFF_VT + 65536].rearrange("(t p c) -> p t c", p=128, c=C))
            nc.vector.memset(vtf4[:, :, C:65], 1.0)
            nc.sync.dma_start(
                statsf[:].rearrange("o (r s) -> o r s", s=2),
                agout[None, :, OFF_ST:OFF_ST + 2])

            # global key maxes and exp biases
            kmax = sp.tile([1, 2], F32, tag="kmax")
            nc.vector.reduce_max(kmax[:], statsf[0:1, :].rearrange("o (r s) -> o s r", s=2),
                                 axis=AX.X)

            with tc.tile_pool(name="bcps", bufs=1, space="PSUM") as bcp:
                def mk_bias(q2, koff):
                    t = sp.tile([1, 1], F32, tag="bias_t" + str(koff))
                    nc.vector.tensor_add(t[:], q2[:], kmax[:, koff:koff + 1])
                    nc.vector.tensor_scalar_mul(t[:], t[:], -0.5)
                    ps = bcp.tile([128, 1], F32, tag="bias_ps" + str(koff))
                    nc.tensor.matmul(ps[:], ones_st[0:1, 0:128], t[0:1, 0:1],
                                     start=True, stop=True)
                    col = cp.tile([128, 1], F32, tag="bias_col" + str(koff))
                    nc.vector.tensor_copy(col[:], ps[:])
                    return col

                bias_x = mk_bias(q2x, 0)
                bias_g = mk_bias(q2g, 1)

            # ---- attention (guide first, then x) ----
            ong = bp.tile([C, QN], F32)    # raw guide_out (masked, unscaled)
            ocx = bp.tile([C, QN], F32)    # gamma * x_out (masked)

            with (
                tc.tile_pool(name="aps_s", bufs=2, space="PSUM") as pss,
                tc.tile_pool(name="aps_o", bufs=2, space="PSUM") as pso,
                tc.tile_pool(name="atp", bufs=3) as atp,
            ):
                for (q_t, kf_t, bias_c, dst, gscale) in (
                    (gqx, gkf, bias_g, ong, None),
                    (qx, kf, bias_x, ocx, gam1_s),
                ):
                    for h in range(2):
                        o = pso.tile([65, HALF], F32, tag="o_ps")
                        for t in range(NT):
                            s = pss.tile([128, HALF], F32, tag="s_ps")
                            nc.tensor.matmul(s[:, 0:512], kf_t[:, 128 * t:128 * (t + 1)],
                                             q_t[:, HALF * h:HALF * h + 512],
                                             start=True, stop=True)
                            nc.tensor.matmul(s[:, 512:HALF], kf_t[:, 128 * t:128 * (t + 1)],
                                             q_t[:, HALF * h + 512:HALF * (h + 1)],
                                             start=True, stop=True)
                            at = atp.tile([128, HALF], F32, tag="at")
                            nc.scalar.activation(at[:], s[:], AF.Exp, bias=bias_c[:, 0:1],
                                                 scale=1.0)
                            nc.tensor.matmul(o[:, 0:512], vtf4[:, t, :], at[:, 0:512],
                                             start=(t == 0), stop=(t == NT - 1))
                            nc.tensor.matmul(o[:, 512:HALF], vtf4[:, t, :], at[:, 512:HALF],
                                             start=(t == 0), stop=(t == NT - 1))
                        rs = sp.tile([1, HALF], F32, tag="rs")
                        nc.vector.tensor_copy(rs[:], o[C:65, :])
                        rc = sp.tile([1, HALF], F32, tag="rc")
                        nc.vector.reciprocal(rc[:], rs[:])
                        nc.vector.tensor_mul(rc[:], rc[:], maskq_s[0:1, HALF * h:HALF * (h + 1)])
                        if gscale is not None:
                            nc.vector.tensor_scalar_mul(rc[:], rc[:], gscale[0:1, 0:1])
                        rbp = pss.tile([128, HALF], F32, tag="s_ps")
                        nc.tensor.matmul(rbp[0:C, 0:512], ones_st[0:1, 0:C],
                                         rc[0:1, 0:512], start=True, stop=True)
                        nc.tensor.matmul(rbp[0:C, 512:HALF], ones_st[0:1, 0:C],
                                         rc[0:1, 512:HALF], start=True, stop=True)
                        rb = sp.tile([C, HALF], F32, tag="rb")
                        nc.vector.tensor_copy(rb[:], rbp[0:C, :])
                        nc.vector.tensor_mul(dst[:, HALF * h:HALF * (h + 1)], o[0:C, :], rb[:])

            # ---- combine + conv tail ----
            oc = bp.tile([C, QN], F32)
            talbum = bp.tile([C, QN], F32)
            nc.vector.tensor_scalar_mul(talbum[:], ong[:], al64_s[:, 0:1])
            nc.vector.tensor_add(oc[:], ocx[:], talbum[:])

            lks = bp.tile([C, QROWS * PW], F32)
            nc.vector.memset(lks[:], 0.0)
            lks3 = lks[:].rearrange("c (r w) -> c r w", w=PW)
            oc3 = oc[:].rearrange("c (r w) -> c r w", w=W)
            nc.vector.tensor_scalar_mul(talbum[:], oc[:], 0.1)
            nc.vector.tensor_max(lks3[:, :, 1:1 + W], oc3[:],
                                 talbum[:].rearrange("c (r w) -> c r w", w=W))

            c1s = bp.tile([C, QROWS * PW], F32)
            nc.vector.memset(c1s[:], 0.0)
            c1s3 = c1s[:].rearrange("c (r w) -> c r w", w=PW)
            mc13 = mc1_s[:].rearrange("c (r w) -> c r w", w=W)

            with tc.tile_pool(name="beps", bufs=3, space="PSUM") as bps:
                # broadcast the c1 row mask across partitions via PE
                for seg, cwid in ((0, 512), (1, 512), (2, 128)):
                    ps = bps.tile([C, 512], F32, tag="be_ps")
                    nc.tensor.matmul(ps[:, 0:cwid], ones_st[0:1, 0:C],
                                     mrow32_s[0:1, 512 * seg:512 * seg + cwid],
                                     start=True, stop=True)
                    nc.vector.tensor_copy(mc1_s[:, 512 * seg:512 * seg + cwid],
                                          ps[:, 0:cwid])

                # c1 on slab rows [1,19)
                for r0, nr in ((1, 8), (9, 8), (17, 2)):
                    ps = bps.tile([C, 512], F32, tag="be_ps")
                    for dy in range(3):
                        for dx in range(3):
                            nc.tensor.matmul(
                                ps[:, 0:nr * W],
                                c1w_s[:, (dy * 3 + dx) * C:(dy * 3 + dx + 1) * C],
                                lks3[:, r0 + dy - 1:r0 + dy - 1 + nr, dx:dx + W],
                                start=(dy == 0 and dx == 0), stop=(dy == 2 and dx == 2))
                    tmp = sp.tile([C, 512], F32, tag="c1_tmp")
                    nc.vector.tensor_scalar_add(tmp[:, 0:nr * W], ps[:, 0:nr * W], c1b_s[:, 0:1])
                    tmp2 = sp.tile([C, 512], F32, tag="c1_tmp2")
                    nc.vector.tensor_scalar_mul(tmp2[:, 0:nr * W], tmp[:, 0:nr * W], 0.1)
                    nc.vector.tensor_max(tmp[:, 0:nr * W], tmp[:, 0:nr * W], tmp2[:, 0:nr * W])
                    nc.vector.tensor_mul(
                        c1s3[:, r0:r0 + nr, 1:1 + W],
                        tmp[:, 0:nr * W].rearrange("c (r w) -> c r w", w=W),
                        mc13[:, r0 - 1:r0 - 1 + nr, :])

                # c2 on slab rows [2,18) -> branch [C, KN]
                branch = bp.tile([C, KN], F32)
                for r0, nr in ((2, 8), (10, 8)):
                    ps = bps.tile([C, 512], F32, tag="be_ps")
                    for dy in range(3):
                        for dx in range(3):
                            nc.tensor.matmul(
                                ps[:, 0:nr * W],
                                c2w_s[:, (dy * 3 + dx) * C:(dy * 3 + dx + 1) * C],
                                c1s3[:, r0 + dy - 1:r0 + dy - 1 + nr, dx:dx + W],
                                start=(dy == 0 and dx == 0), stop=(dy == 2 and dx == 2))
                    nc.vector.tensor_scalar_add(branch[:, (r0 - 2) * W:(r0 - 2 + nr) * W],
                                                ps[:, 0:nr * W], c2b_s[:, 0:1])

                # sc 1x1 on oc rows [2,18), final = branch + sc * guide_out (fp16 out)
                finalv = bp.tile([C, KN], F16)
                for c0 in (0, 512):
                    ps = bps.tile([C, 512], F32, tag="be_ps")
                    nc.tensor.matmul(ps[:], scw_s[:], oc[:, 2 * W + c0:2 * W + c0 + 512],
                                     start=True, stop=True)
                    tmp = sp.tile([C, 512], F32, tag="sc_tmp")
                    nc.vector.tensor_scalar_add(tmp[:], ps[:], scb_s[:, 0:1])
                    nc.vector.tensor_mul(tmp[:], tmp[:], ong[:, 2 * W + c0:2 * W + c0 + 512])
                    nc.vector.tensor_add(finalv[:, c0:c0 + 512], branch[:, c0:c0 + 512], tmp[:])

                nc.sync.dma_start(out_d[:], finalv[:])

    nc.compile()
    return nc


def _host_inputs(inputs):
    """Build the 8 per-core input maps from the full problem inputs."""
    x = np.asarray(inputs["x"], np.float32)
    guide = np.asarray(inputs["guide"], np.float32)
    lin_w = float(np.asarray(inputs["lin_w"]))
    lin_b = float(np.asarray(inputs["lin_b"]))
    coord_w = np.asarray(inputs["coord_w"], np.float32)   # (64, 66, 3, 3)
    gamma = float(np.asarray(inputs["gamma"]).reshape(-1)[0])
    alpha = float(np.asarray(inputs["alpha"]).reshape(-1)[0])

    # channel attention: sigmoid(lw*leaky(lw*mean + lb) + lb), 64 scalars/batch
    def aw_host(t):
        p = t.mean(axis=(2, 3), dtype=np.float64) * lin_w + lin_b
        h = np.where(p > 0, p, 0.1 * p) * lin_w + lin_b
        return (1.0 / (1.0 + np.exp(-h))).astype(np.float32)   # (B, C)

    awx = aw_host(x)
    awg = aw_host(guide)

    # coordinate channels
    xx = (np.arange(W, dtype=np.float32) / (W - 1)) * 2 - 1
    yy = (np.arange(H, dtype=np.float32) / (H - 1)) * 2 - 1

    def taps(w):  # (O, I, 3, 3) -> [I, 9*O], tap-major
        o, i = w.shape[0], w.shape[1]
        out = np.zeros((i, 9 * o), np.float32)
        for dy in range(3):
            for dx in range(3):
                out[:, (dy * 3 + dx) * o:(dy * 3 + dx + 1) * o] = w[:, :, dy, dx].T
        return out

    wT = lambda k: np.asarray(inputs[k], np.float32).T
    bc = lambda k: np.asarray(inputs[k], np.float32).reshape(C)

    vtwb = np.concatenate(
        [wT("xv_w"), np.asarray(inputs["xv_b"], np.float32).reshape(1, C)], axis=0)

    blob = np.concatenate([
        taps(coord_w).ravel(),
        taps(np.asarray(inputs["c1_w"], np.float32)).ravel(),
        taps(np.asarray(inputs["c2_w"], np.float32)).ravel(),
        wT("xq_w").ravel(), wT("xk_w").ravel(),
        wT("gq_w").ravel(), wT("gk_w").ravel(),
        vtwb.ravel(), wT("sc_w").ravel(),
    ]).astype(np.float16).reshape(4, WSH)

    bvec_b = []
    for b in range(B):
        v = np.zeros((C, 12), np.float16)
        v[:, 0] = bc("xq_b"); v[:, 1] = bc("xk_b")
        v[:, 2] = bc("gq_b"); v[:, 3] = bc("gk_b")
        v[:, 4] = bc("c1_b"); v[:, 5] = bc("c2_b"); v[:, 6] = bc("sc_b")
        v[:, 7] = awx[b]; v[:, 8] = awg[b]
        v[:, 9] = alpha; v[:, 10] = gamma
        bvec_b.append(v)

    in_maps = []
    for i in range(8):
        b, j = divmod(i, 4)
        pkv = np.zeros(PKTOT, np.float16)
        cs = pkv[OFF_SLABX:OFF_SLABG].reshape(66, SLABR, PW)
        gs = pkv[OFF_SLABG:OFF_W].reshape(66, SLABR, PW)

        r_lo = 16 * j - 3                      # slab image rows [r_lo, r_lo+22)
        lo, hi = max(0, r_lo), min(H, r_lo + SLABR)
        s0, s1 = lo - r_lo, hi - r_lo
        cs[0:C, s0:s1, 1:1 + W] = x[b, :, lo:hi, :]
        gs[0:C, s0:s1, 1:1 + W] = guide[b, :, lo:hi, :]
        cs[C, s0:s1, 1:1 + W] = xx
        cs[C + 1, s0:s1, 1:1 + W] = yy[lo:hi, None]
        gs[C, s0:s1, 1:1 + W] = xx
        gs[C + 1, s0:s1, 1:1 + W] = yy[lo:hi, None]

        pkv[OFF_W:OFF_BVEC] = blob[j]
        pkv[OFF_BVEC:OFF_MQ] = bvec_b[b].ravel()

        q_lo = 16 * j - 2
        mq = pkv[OFF_MQ:OFF_MC1].reshape(QROWS, W)
        mq[max(0, -q_lo):QROWS - max(0, q_lo + QROWS - H)] = 1.0
        c_lo = 16 * j - 1
        mc = pkv[OFF_MC1:PKTOT].reshape(18, W)
        mc[max(0, -c_lo):18 - max(0, c_lo + 18 - H)] = 1.0

        in_maps.append(dict(pk=pkv))
    return in_maps


def kernel(**inputs):
    if "nc" not in _CACHE:
        _CACHE["nc"] = _build_program()
    nc = _CACHE["nc"]
    in_maps = _host_inputs(inputs)
    res = run_bass_kernel_spmd(nc, in_maps, core_ids=list(range(8)))
    out = np.zeros((B, C, H, W), np.float32)
    for i in range(8):
        b, j = divmod(i, 4)
        out[b, :, 16 * j:16 * j + 16, :] = (
            res.results[i]["out"].astype(np.float32).reshape(C, KROWS, W))
    return out
